# revision 7
# baseline (speedup 1.0000x reference)
"""CapsuleMaxPooling Trainium2 kernel.

Problem: inp [B=32, C=32, H=64, W=64, D=8] f32, kernel_size k=2.
For each 2x2 spatial window pick the capsule vector (length D=8) with the
largest squared L2 norm (first-max tie-break) -> out [B, C, 32, 32, 8].

Strategy (fully data-parallel, shard B across 8 cores, 4 batches/core):
  - View the per-core shard as rows r=(b, c, hk) of 1024 contiguous floats
    = (dh, wk, dw, d) [2*32*2*8]: both H-rows of all windows in that row.
  - ACT: sq = x^2 (Square activation)
  - DVE: norms = grouped reduce_sum over d (groups of 8)
  - masks: 2-level tournament with >= tie-breaks (matches first-argmax):
        m1 = nA >= nB ; m2 = nC >= nD ; m3 = max(nA,nB) >= max(nC,nD)
        wA = m1*m3 ; wB = m3-wA ; wC = m2*(1-m3)
  - select: base-copy candidate D (ACT), then 3x copy_predicated (DVE)
    with masks broadcast over d via stride-0 APs.
  - HWDGE (nc.sync) DMAs; contiguous 4KB-per-partition chunks.
"""

import numpy as np

try:
    import concourse.bass as bass
except ImportError:  # pragma: no cover
    import sys

    sys.path.insert(0, "/opt/trn_rl_repo")
    import concourse.bass as bass

from concourse import bacc, mybir
from concourse.bass_utils import run_bass_kernel_spmd
from concourse.tile import TileContext

P = 128
N_CORES = 8
ROW_W = 1024  # (dh=2) * (wk=32) * (dw=2) * (d=8)
OUT_W = 256  # (wk=32) * (d=8)


def _ap_bc8(w):
    """View mask tile w [P, TB, 32] as [P, TB, 32, 8] via stride-0 inner dim."""
    a = w[:]
    return bass.AP(tensor=a.tensor, offset=a.offset, ap=[*a.ap, [0, 8]])


def build_nc(R=4096, TB=8, mask_engine="vector"):
    """Build the per-core Bass program. R = rows (b,c,hk) per core."""
    f32 = mybir.dt.float32
    u8 = mybir.dt.uint8
    nc = bacc.Bacc(None, target_bir_lowering=False)
    x = nc.dram_tensor("x", [R, ROW_W], f32, kind="ExternalInput")
    y = nc.dram_tensor("y", [R, OUT_W], f32, kind="ExternalOutput")
    ntiles = R // P
    assert R % P == 0 and ntiles % TB == 0
    nbatch = ntiles // TB

    with TileContext(nc) as tc:
        with (
            tc.tile_pool(name="xp", bufs=2) as xp,
            tc.tile_pool(name="sqp", bufs=2) as sqp,
            tc.tile_pool(name="normp", bufs=2) as normp,
            tc.tile_pool(name="maskp", bufs=2) as maskp,
            tc.tile_pool(name="outp", bufs=2) as outp,
        ):
            me = nc.vector if mask_engine == "vector" else nc.gpsimd
            for t in range(nbatch):
                r0 = t * TB * P
                r1 = r0 + TB * P
                xt = xp.tile([P, TB, ROW_W], f32)
                nc.sync.dma_start(
                    out=xt, in_=x[r0:r1, :].rearrange("(j p) c -> p j c", p=P)
                )

                sq = sqp.tile([P, TB, ROW_W], f32)
                nc.scalar.square(sq, xt)

                norms = normp.tile([P, TB, 128], f32)
                nc.vector.tensor_reduce(
                    norms,
                    sq.rearrange("p j (g d) -> p j g d", d=8),
                    axis=mybir.AxisListType.X,
                    op=mybir.AluOpType.add,
                )

                nr = norms.rearrange("p j (dh wk dw) -> p j dh wk dw", dh=2, dw=2)
                nA = nr[:, :, 0, :, 0]
                nB = nr[:, :, 0, :, 1]
                nC = nr[:, :, 1, :, 0]
                nD = nr[:, :, 1, :, 1]

                m1 = maskp.tile([P, TB, 32], u8)
                me.tensor_tensor(m1, nA, nB, op=mybir.AluOpType.is_ge)
                m2 = maskp.tile([P, TB, 32], u8)
                me.tensor_tensor(m2, nC, nD, op=mybir.AluOpType.is_ge)
                h1 = maskp.tile([P, TB, 32], f32)
                me.tensor_tensor(h1, nA, nB, op=mybir.AluOpType.max)
                h2 = maskp.tile([P, TB, 32], f32)
                me.tensor_tensor(h2, nC, nD, op=mybir.AluOpType.max)
                m3 = maskp.tile([P, TB, 32], u8)
                me.tensor_tensor(m3, h1, h2, op=mybir.AluOpType.is_ge)
                wA = maskp.tile([P, TB, 32], u8)
                me.tensor_tensor(wA, m1, m3, op=mybir.AluOpType.mult)
                wB = maskp.tile([P, TB, 32], u8)
                me.tensor_tensor(wB, m3, wA, op=mybir.AluOpType.subtract)
                t2 = maskp.tile([P, TB, 32], u8)
                me.tensor_tensor(t2, m2, m3, op=mybir.AluOpType.mult)
                wC = maskp.tile([P, TB, 32], u8)
                me.tensor_tensor(wC, m2, t2, op=mybir.AluOpType.subtract)

                xr = xt.rearrange("p j (dh wk dw d) -> p j dh wk dw d", dh=2, dw=2, d=8)
                Av = xr[:, :, 0, :, 0, :]
                Bv = xr[:, :, 0, :, 1, :]
                Cv = xr[:, :, 1, :, 0, :]
                Dv = xr[:, :, 1, :, 1, :]

                ot = outp.tile([P, TB, 32, 8], f32)
                nc.scalar.copy(ot, Dv)
                nc.vector.copy_predicated(ot, _ap_bc8(wC), Cv)
                nc.vector.copy_predicated(ot, _ap_bc8(wB), Bv)
                nc.vector.copy_predicated(ot, _ap_bc8(wA), Av)

                nc.sync.dma_start(
                    out=y[r0:r1, :].rearrange("(j p) c -> p j c", p=P),
                    in_=ot.rearrange("p j w d -> p j (w d)"),
                )
    nc.compile()
    return nc


_NC_CACHE = {}


def _get_nc(R, TB):
    key = (R, TB)
    if key not in _NC_CACHE:
        _NC_CACHE[key] = build_nc(R, TB)
    return _NC_CACHE[key]


def kernel(inp, kernel_size):
    inp = np.asarray(inp)
    k = int(np.asarray(kernel_size))
    assert k == 2, f"kernel hardcoded for kernel_size=2, got {k}"
    B, C, H, W, D = inp.shape
    assert (B, C, H, W, D) == (32, 32, 64, 64, 8), inp.shape
    Hk, Wk = H // k, W // k

    bs = B // N_CORES  # 4 batches per core
    R = bs * C * Hk  # 4096 rows per core
    nc = _get_nc(R, TB=8)

    in_maps = []
    for c in range(N_CORES):
        shard = np.ascontiguousarray(inp[c * bs : (c + 1) * bs]).reshape(R, ROW_W)
        in_maps.append({"x": shard})

    res = run_bass_kernel_spmd(nc, in_maps, list(range(N_CORES)))
    out = np.concatenate(
        [r["y"].reshape(bs, C, Hk, Wk, D) for r in res.results], axis=0
    )
    return out
